# revision 1
# baseline (speedup 1.0000x reference)
"""Multi-head attention (B=4, N=2048, C=768, H=12) on 8 TRN2 NeuronCores.

Sharding: 4 batches x 2 head-groups (6 heads each); core = 2*b + g.
Per core:
  - qT/kT [64,2048] per head and v [2048,64] per head from host-pre-transposed xT
  - flash-style attention on transposed-S tiles:
      S^T(m,n) = kT.T @ qT   (PE, bf16)
      P^T = exp(S^T/8)       (ACT, -> bf16)
      o^T = [v|1].T @ P^T    (PE, bf16; ones column accumulates softmax row-sums)
  - normalize columns of o^T via reciprocal + K=1 broadcast matmul (plain f32)
  - AllGather normalized aoT (bf16) between pair cores, chunked over two
    1024-column halves so the collective overlaps the other half's attention
  - each core projects full aoT onto its half of w_out columns -> y [2048,384]
Host only concatenates the 8 column-slices (no host math).
"""

import sys

sys.path.insert(0, "/opt/trn_rl_repo")

import ml_dtypes
import numpy as np

import concourse.bass as bass
import concourse.mybir as mybir
from concourse import bacc, tile
from concourse.bass_utils import run_bass_kernel_spmd

F32 = mybir.dt.float32
BF16 = mybir.dt.bfloat16

B, N, C, H, D = 4, 2048, 768, 12, 64
G = 2               # head groups (tensor-parallel dim)
HPC = H // G        # heads per core = 6
KC = HPC * D        # per-core head width = 384
CT = C // 128       # contraction tiles over C = 6
NT = N // 128       # 128-row seq tiles = 16
SCALE = D ** -0.5


def _build():
    nc = bacc.Bacc(None, num_devices=8)

    xT_d = nc.declare_dram_parameter("xT", [C, N], BF16, isOutput=False)
    wq_d = nc.declare_dram_parameter("wq", [C, KC], BF16, isOutput=False)
    wk_d = nc.declare_dram_parameter("wk", [C, KC], BF16, isOutput=False)
    wv_d = nc.declare_dram_parameter("wv", [C, KC], BF16, isOutput=False)
    wo_d = nc.declare_dram_parameter("wo", [C, KC], BF16, isOutput=False)
    bb_d = nc.declare_dram_parameter("bb", [128, KC], F32, isOutput=False)
    y_d = nc.declare_dram_parameter("y", [N, KC], F32, isOutput=True)

    with tile.TileContext(nc) as tc:
        with (
            tc.tile_pool(name="wpool", bufs=1) as wpool,
            tc.tile_pool(name="xpool", bufs=1) as xpool,
            tc.tile_pool(name="seq", bufs=1) as seq,
            tc.tile_pool(name="work", bufs=3) as work,
            tc.tile_pool(name="small", bufs=2) as small,
            tc.tile_pool(name="psum", bufs=2, space="PSUM") as psum,
            tc.tile_pool(name="dram", bufs=1, space="DRAM") as dram,
        ):
            # ---- load inputs, convert to bf16 on DVE ----
            with nc.named_scope("load"):
                wq_sb = wpool.tile([128, CT, KC], BF16)
                wk_sb = wpool.tile([128, CT, KC], BF16)
                wv_sb = wpool.tile([128, CT, KC], BF16)
                wo_sb = wpool.tile([128, CT, KC], BF16)
                bb_sb = wpool.tile([128, KC], F32)
                xT_sb = xpool.tile([128, CT, N], BF16)
                # wq + the first 512-col slice of xT land first so the first
                # qk psum group unblocks early; the rest stays as wide DMAs
                for ct in range(CT):
                    nc.sync.dma_start(wq_sb[:, ct, :], wq_d[ct * 128:(ct + 1) * 128, :])
                for ct in range(CT):
                    nc.sync.dma_start(xT_sb[:, ct, 0:512], xT_d[ct * 128:(ct + 1) * 128, 0:512])
                for wd, wsb in ((wk_d, wk_sb), (wv_d, wv_sb)):
                    for ct in range(CT):
                        nc.sync.dma_start(wsb[:, ct, :], wd[ct * 128:(ct + 1) * 128, :])
                for ct in range(CT):
                    nc.sync.dma_start(xT_sb[:, ct, 512:N], xT_d[ct * 128:(ct + 1) * 128, 512:N])
                for ct in range(CT):
                    nc.sync.dma_start(wo_sb[:, ct, :], wo_d[ct * 128:(ct + 1) * 128, :])
                nc.sync.dma_start(bb_sb[:], bb_d[:])

            # ---- tiles for qT/kT pairs, v (with ones column), attention output ----
            qT_sb = [seq.tile([128, N], BF16, name=f"qT{t}", tag=f"qT{t}") for t in range(3)]
            kT_sb = [seq.tile([128, N], BF16, name=f"kT{t}", tag=f"kT{t}") for t in range(3)]
            v_sb = seq.tile([128, NT * HPC * 65], BF16, tag="v")
            # ones column at offset 64 of every 65-wide block (softmax row-sum trick)
            nc.vector.memset(v_sb.rearrange("p (b s) -> p b s", s=65)[:, :, 64], 1.0)
            ao_sb = [seq.tile([128, N], BF16, name=f"ao{t}", tag=f"ao{t}") for t in range(3)]
            ones_sb = small.tile([1, 64], BF16, bufs=1)
            nc.vector.memset(ones_sb[:], 1.0)
            ag_bounce = [
                dram.tile([KC, 1024], BF16, name=f"ag_in{nic}") for nic in range(2)
            ]
            def emit_qk_pair(hp):
                with nc.named_scope("qkv"):
                    for wsb, dst in ((wq_sb, qT_sb[hp]), (wk_sb, kT_sb[hp])):
                        for ni in range(4):
                            qk_ps = psum.tile([128, 512], F32, name="qk_ps", tag="mm")
                            for ct in range(CT):
                                nc.tensor.matmul(
                                    qk_ps[:],
                                    wsb[:, ct, hp * 128:(hp + 1) * 128],
                                    xT_sb[:, ct, ni * 512:(ni + 1) * 512],
                                    start=(ct == 0), stop=(ct == CT - 1),
                                )
                            nc.vector.tensor_copy(dst[:, ni * 512:(ni + 1) * 512], qk_ps[:])

            def emit_v():
                with nc.named_scope("qkv"):
                    for mj in range(NT):
                        v_ps = psum.tile([128, KC], F32, name="v_ps", tag="mm")
                        for ct in range(CT):
                            nc.tensor.matmul(
                                v_ps[:],
                                xT_sb[:, ct, mj * 128:(mj + 1) * 128],
                                wv_sb[:, ct, :],
                                start=(ct == 0), stop=(ct == CT - 1),
                            )
                        for h in range(HPC):
                            nc.vector.tensor_copy(
                                v_sb[:, (mj * HPC + h) * 65:(mj * HPC + h) * 65 + 64],
                                v_ps[:, h * 64:(h + 1) * 64],
                            )

            def emit_attn_pair(nic, hp, half):
                # Two heads of a pair interleaved at the mj level: their sT
                # matmuls target disjoint PE row groups (0-63 / 64-127) so they
                # execute concurrently in the array, double occupancy (keeps
                # HAM at K=8/8) and hide LDWEIGHTS under the other group's MM.
                # 512-wide n-chunks so PSUM fits 2 live sT + 2 oT accumulators.
                nc5 = nic * 2 + half
                col = nc5 * 512
                with nc.named_scope(f"attn{nic}"):
                    t = hp
                    kT_h, qT_h = kT_sb[t], qT_sb[t]
                    oT = [
                        psum.tile([65, 512], F32, name=f"oT{i}", tag="oT")
                        for i in range(2)
                    ]
                    for mj in range(NT):
                        # both heads' S^T tiles share one 2-bank psum tile so a
                        # single 1024-wide ACT covers both heads' exp
                        sT = psum.tile([128, 1024], F32, name="sT", tag="sT", bufs=2)
                        for i in range(2):  # i = head within pair, row group i*64
                            po = i * 64
                            nc.tensor.matmul(
                                sT[:, i * 512:(i + 1) * 512],
                                kT_h[po:po + 64, mj * 128:(mj + 1) * 128],
                                qT_h[po:po + 64, col:col + 512],
                                start=True, stop=True,
                            )
                        pT = work.tile([128, 1024], BF16, name="pT", tag="pT", bufs=6)
                        nc.scalar.activation(
                            pT[:], sT[:], mybir.ActivationFunctionType.Exp, scale=SCALE,
                        )
                        for i in range(2):
                            h = hp * 2 + i
                            vblk = v_sb[:, (mj * HPC + h) * 65:(mj * HPC + h) * 65 + 65]
                            nc.tensor.matmul(
                                oT[i][:], vblk, pT[:, i * 512:(i + 1) * 512],
                                start=(mj == 0), stop=(mj == NT - 1),
                            )
                    # Normalize: stash unnormalized output (frees the oT banks for
                    # the next chunk), fast-approx reciprocal of the row-sums
                    # (~5x faster than exact; 18-bit accuracy is plenty), K=1
                    # broadcast matmul, in-place scale, then ship the slice.
                    for i in range(2):
                        po = i * 64
                        ao_slice = ao_sb[t][po:po + 64, col:col + 512]
                        nc.vector.tensor_copy(ao_slice, oT[i][0:64, :])
                        r_row = small.tile([1, 512], F32, name="r_row", tag="r_row")
                        nc.vector.tensor_copy(r_row[:], oT[i][64:65, :])
                        rinv = small.tile([1, 512], F32, name="rinv", tag="rinv")
                        nc.vector.reciprocal_approx_fast(rinv[:], r_row[:])
                        rb_row = small.tile([1, 512], BF16, name="rb_row", tag="rb_row", bufs=4)
                        nc.vector.tensor_copy(rb_row[:], rinv[:])
                        rb_ps = psum.tile([64, 512], F32, name="rb_ps", tag="mm")
                        nc.tensor.matmul(rb_ps[:], ones_sb[:], rb_row[:], start=True, stop=True)
                        nc.vector.tensor_mul(ao_slice, ao_slice, rb_ps[:])
                        nc.gpsimd.dma_start(
                            ag_bounce[nic][t * 128 + po: t * 128 + po + 64, half * 512:half * 512 + 512],
                            ao_slice,
                        )

            # Interleave the dense qkv GEMMs between attention pair-chunks to
            # keep the PE warm and its gaps filled.
            emit_qk_pair(0)
            emit_v()
            emit_attn_pair(0, 0, 0)
            emit_attn_pair(0, 0, 1)
            emit_qk_pair(1)
            emit_attn_pair(0, 1, 0)
            emit_attn_pair(0, 1, 1)
            emit_qk_pair(2)
            emit_attn_pair(0, 2, 0)
            emit_attn_pair(0, 2, 1)
            for nic in range(2):
                if nic == 0:
                    # first nic1 chunk keeps ACT fed while ship0 runs
                    emit_attn_pair(1, 0, 0)
                if nic == 1:
                    emit_attn_pair(1, 0, 1)
                    for hp in range(1, 3):
                        emit_attn_pair(1, hp, 0)
                        emit_attn_pair(1, hp, 1)

                # ship this 1024-column chunk while the other chunk computes
                # (per-head slices were DMA'd into ag_bounce as they finished)
                with nc.named_scope(f"ag{nic}"):
                    ag_in = ag_bounce[nic]
                    ag_out = dram.tile([C, 1024], BF16, name=f"ag_out{nic}")
                    nc.gpsimd.collective_compute(
                        "AllGather",
                        mybir.AluOpType.bypass,
                        replica_groups=[[0, 1], [2, 3], [4, 5], [6, 7]],
                        ins=[ag_in.opt()],
                        outs=[ag_out.opt()],
                    )

                with nc.named_scope(f"proj{nic}"):
                    aoF = []
                    for kt in range(CT):
                        aof = work.tile([128, 1024], BF16, name=f"aoF{kt}", tag=f"aoF{kt}", bufs=1)
                        nc.sync.dma_start(aof[:], ag_out[kt * 128:(kt + 1) * 128, :])
                        aoF.append(aof)
                    for njl in range(8):
                        nj = nic * 8 + njl
                        y_ps = psum.tile([128, KC], F32, name="y_ps", tag="mm")
                        for kt in range(CT):
                            nc.tensor.matmul(
                                y_ps[:],
                                aoF[kt][:, njl * 128:(njl + 1) * 128],
                                wo_sb[:, kt, :],
                                start=(kt == 0), stop=(kt == CT - 1),
                            )
                        y_sb = work.tile([128, KC], F32, name="y_sb", tag="y")
                        nc.vector.tensor_add(y_sb[:], y_ps[:], bb_sb[:])
                        nc.sync.dma_start(y_d[nj * 128:(nj + 1) * 128, :], y_sb[:])

    nc.finalize()
    return nc


_NC = None
LAST_RESULTS = None


def _get_nc():
    global _NC
    if _NC is None:
        _NC = _build()
    return _NC


def kernel(x, w_qkv, w_out, b_out, _trace=False):
    global LAST_RESULTS
    nc = _get_nc()

    x = np.asarray(x, dtype=np.float32)
    w_qkv = np.asarray(w_qkv, dtype=np.float32)
    w_out = np.asarray(w_out, dtype=np.float32)
    b_out = np.asarray(b_out, dtype=np.float32)

    bf16 = ml_dtypes.bfloat16
    in_maps = []
    for c in range(8):
        b, g = c // 2, c % 2
        s = g * KC
        in_maps.append({
            "xT": np.ascontiguousarray(x[b].T).astype(bf16),
            "wq": np.ascontiguousarray(w_qkv[:, s:s + KC]).astype(bf16),
            "wk": np.ascontiguousarray(w_qkv[:, C + s:C + s + KC]).astype(bf16),
            "wv": np.ascontiguousarray(w_qkv[:, 2 * C + s:2 * C + s + KC]).astype(bf16),
            "wo": np.ascontiguousarray(w_out[:, s:s + KC]).astype(bf16),
            "bb": np.tile(b_out[s:s + KC], (128, 1)),
        })

    res = run_bass_kernel_spmd(nc, in_maps, core_ids=list(range(8)), trace=_trace)
    LAST_RESULTS = res

    out = np.empty((B, N, C), dtype=np.float32)
    for c in range(8):
        b, g = c // 2, c % 2
        out[b, :, g * KC:(g + 1) * KC] = res.results[c]["y"]
    return out



# revision 4
# speedup vs baseline: 1.2290x; 1.2290x over previous
"""Multi-head attention (B=4, N=2048, C=768, H=12) on 8 TRN2 NeuronCores.

Sharding: 4 batches x 2 head-groups (6 heads each); core = 2*b + g.
Per core:
  - qT/kT [64,2048] per head and v [2048,64] per head from host-pre-transposed xT
  - flash-style attention on transposed-S tiles:
      S^T(m,n) = kT.T @ qT   (PE, bf16, two heads paired in disjoint row groups)
      P^T = exp(S^T/8)       (ACT, -> bf16)
      o^T = [v|1].T @ P^T    (PE, bf16; ones column accumulates softmax row-sums)
  - normalize columns of o^T via reciprocal + K=1 broadcast matmul
  - AllGather normalized aoT (bf16) between pair cores; chunked 1024 + 512 + 512
    so collectives and the projection overlap attention of later chunks
  - each core projects full aoT onto its half of w_out columns -> y [2048,384]

Scheduling: the qkv GEMM + projection GEMM work is cut into single-PSUM-group
units and drained one group per attention mj-iteration ("background work"), so
the PE stream stays dense while ACT paces the attention inner loop. Attention
starts as soon as the first k/q/v groups exist instead of after the full qkv.
Host only concatenates the 8 column-slices (no host math).
"""

import sys

sys.path.insert(0, "/opt/trn_rl_repo")

import ml_dtypes
import numpy as np

import concourse.bass as bass
import concourse.mybir as mybir
from concourse import bacc, tile
from concourse.bass_utils import run_bass_kernel_spmd

F32 = mybir.dt.float32
BF16 = mybir.dt.bfloat16

B, N, C, H, D = 4, 2048, 768, 12, 64
G = 2               # head groups (tensor-parallel dim)
HPC = H // G        # heads per core = 6
KC = HPC * D        # per-core head width = 384
CT = C // 128       # contraction tiles over C = 6
NT = N // 128       # 128-row seq tiles = 16
SCALE = D ** -0.5


def _build():
    nc = bacc.Bacc(None, num_devices=8)

    xT_d = nc.declare_dram_parameter("xT", [C, N], BF16, isOutput=False)
    wq_d = nc.declare_dram_parameter("wq", [C, KC], BF16, isOutput=False)
    wk_d = nc.declare_dram_parameter("wk", [C, KC], BF16, isOutput=False)
    wv_d = nc.declare_dram_parameter("wv", [C, KC], BF16, isOutput=False)
    wo_d = nc.declare_dram_parameter("wo", [C, KC], BF16, isOutput=False)
    bb_d = nc.declare_dram_parameter("bb", [128, KC], F32, isOutput=False)
    y_d = nc.declare_dram_parameter("y", [N, KC], F32, isOutput=True)

    with tile.TileContext(nc) as tc:
        with (
            tc.tile_pool(name="wpool", bufs=1) as wpool,
            tc.tile_pool(name="xpool", bufs=1) as xpool,
            tc.tile_pool(name="seq", bufs=1) as seq,
            tc.tile_pool(name="work", bufs=3) as work,
            tc.tile_pool(name="small", bufs=2) as small,
            tc.tile_pool(name="psum", bufs=2, space="PSUM") as psum,
            tc.tile_pool(name="dram", bufs=1, space="DRAM") as dram,
        ):
            # ---- input DMAs (host supplies bf16) ----
            with nc.named_scope("load"):
                wq_sb = wpool.tile([128, CT, KC], BF16)
                wk_sb = wpool.tile([128, CT, KC], BF16)
                wv_sb = wpool.tile([128, CT, KC], BF16)
                wo_sb = wpool.tile([128, CT, KC], BF16)
                bb_sb = wpool.tile([128, KC], F32)
                xT_sb = xpool.tile([128, CT, N], BF16)
                # wq + wk + wv + first 512-col slice of xT land first so the
                # first q/k/v psum groups unblock early
                for ct in range(CT):
                    nc.sync.dma_start(wq_sb[:, ct, :], wq_d[ct * 128:(ct + 1) * 128, :])
                for ct in range(CT):
                    nc.sync.dma_start(xT_sb[:, ct, 0:512], xT_d[ct * 128:(ct + 1) * 128, 0:512])
                for wd, wsb in ((wk_d, wk_sb), (wv_d, wv_sb)):
                    for ct in range(CT):
                        nc.sync.dma_start(wsb[:, ct, :], wd[ct * 128:(ct + 1) * 128, :])
                for ct in range(CT):
                    nc.sync.dma_start(xT_sb[:, ct, 512:N], xT_d[ct * 128:(ct + 1) * 128, 512:N])
                for ct in range(CT):
                    nc.sync.dma_start(wo_sb[:, ct, :], wo_d[ct * 128:(ct + 1) * 128, :])
                nc.sync.dma_start(bb_sb[:], bb_d[:])

            # ---- persistent tiles ----
            qT_sb = [seq.tile([128, N], BF16, name=f"qT{t}", tag=f"qT{t}") for t in range(3)]
            kT_sb = [seq.tile([128, N], BF16, name=f"kT{t}", tag=f"kT{t}") for t in range(3)]
            v_sb = seq.tile([128, NT * HPC * 65], BF16, tag="v")
            # ones column at offset 64 of every 65-wide block (softmax row-sum trick)
            nc.vector.memset(v_sb.rearrange("p (b s) -> p b s", s=65)[:, :, 64], 1.0)
            ao_sb = [seq.tile([128, N], BF16, name=f"ao{t}", tag=f"ao{t}") for t in range(3)]
            ones_sb = small.tile([1, 64], BF16, bufs=1)
            nc.vector.memset(ones_sb[:], 1.0)
            # AllGather bounce buffers: chunk 0 = cols 0:1024, then 1024:1536, 1536:2048
            ag_in = [
                dram.tile([KC, w], BF16, name=f"ag_in{i}")
                for i, w in enumerate((1024, 512, 512))
            ]
            ag_out = [
                dram.tile([C, w], BF16, name=f"ag_out{i}")
                for i, w in enumerate((1024, 512, 512))
            ]

            # ---- background work units (one PSUM group each) ----
            def qk_group(wsb, dst, hp, ni):
                # qT or kT for head-pair hp, columns ni*512:(ni+1)*512
                with nc.named_scope("qkv"):
                    qk_ps = psum.tile([128, 512], F32, name="qk_ps", tag="mm")
                    for ct in range(CT):
                        nc.tensor.matmul(
                            qk_ps[:],
                            wsb[:, ct, hp * 128:(hp + 1) * 128],
                            xT_sb[:, ct, ni * 512:(ni + 1) * 512],
                            start=(ct == 0), stop=(ct == CT - 1),
                        )
                    nc.vector.tensor_copy(dst[:, ni * 512:(ni + 1) * 512], qk_ps[:])

            def v_group(mj):
                with nc.named_scope("qkv"):
                    v_ps = psum.tile([128, KC], F32, name="v_ps", tag="mm")
                    for ct in range(CT):
                        nc.tensor.matmul(
                            v_ps[:],
                            xT_sb[:, ct, mj * 128:(mj + 1) * 128],
                            wv_sb[:, ct, :],
                            start=(ct == 0), stop=(ct == CT - 1),
                        )
                    for h in range(HPC):
                        nc.vector.tensor_copy(
                            v_sb[:, (mj * HPC + h) * 65:(mj * HPC + h) * 65 + 64],
                            v_ps[:, h * 64:(h + 1) * 64],
                        )

            aoF = {}  # chunk -> list of CT sbuf tiles holding gathered aoT

            def proj_load(ci):
                # DMA gathered chunk ci back to SBUF (cheap, overlaps)
                w = 1024 if ci == 0 else 512
                with nc.named_scope(f"proj{ci}"):
                    tiles = []
                    for kt in range(CT):
                        t = work.tile([128, w], BF16, name=f"aoF{ci}_{kt}",
                                      tag=f"aoF{kt}", bufs=1)
                        nc.sync.dma_start(t[:], ag_out[ci][kt * 128:(kt + 1) * 128, :])
                        tiles.append(t)
                    aoF[ci] = tiles

            def proj_group(ci, njl):
                # one 128-row block of y within chunk ci's column window
                nj = njl + (0 if ci == 0 else 8 + (ci - 1) * 4)
                with nc.named_scope(f"proj{ci}"):
                    y_ps = psum.tile([128, KC], F32, name="y_ps", tag="mm")
                    for kt in range(CT):
                        nc.tensor.matmul(
                            y_ps[:],
                            aoF[ci][kt][:, njl * 128:(njl + 1) * 128],
                            wo_sb[:, kt, :],
                            start=(kt == 0), stop=(kt == CT - 1),
                        )
                    y_sb = work.tile([128, KC], F32, name="y_sb", tag="y")
                    nc.vector.tensor_add(y_sb[:], y_ps[:], bb_sb[:])
                    nc.sync.dma_start(y_d[nj * 128:(nj + 1) * 128, :], y_sb[:])

            from collections import deque
            bg = deque()

            def drain_bg(n=1):
                for _ in range(n):
                    if bg:
                        bg.popleft()()

            def attn_chunk(hp, c):
                # attention for head-pair hp over query columns c*512:(c+1)*512;
                # ships normalized slices into the right ag bounce buffer.
                ci = 0 if c < 2 else c - 1          # ag chunk index
                coff = c * 512 if c < 2 else 0      # col offset inside ag_in[ci]
                col = c * 512
                with nc.named_scope(f"attn{c}"):
                    t = hp
                    kT_h, qT_h = kT_sb[t], qT_sb[t]
                    oT = [
                        psum.tile([65, 512], F32, name=f"oT{i}", tag="oT")
                        for i in range(2)
                    ]
                    for mj in range(NT):
                        # both heads' S^T tiles share one 2-bank psum tile so a
                        # single 1024-wide ACT covers both heads' exp
                        sT = psum.tile([128, 1024], F32, name="sT", tag="sT", bufs=2)
                        for i in range(2):  # i = head within pair, PE row group i*64
                            po = i * 64
                            nc.tensor.matmul(
                                sT[:, i * 512:(i + 1) * 512],
                                kT_h[po:po + 64, mj * 128:(mj + 1) * 128],
                                qT_h[po:po + 64, col:col + 512],
                                start=True, stop=True,
                            )
                        pT = work.tile([128, 1024], BF16, name="pT", tag="pT", bufs=6)
                        nc.scalar.activation(
                            pT[:], sT[:], mybir.ActivationFunctionType.Exp, scale=SCALE,
                        )
                        for i in range(2):
                            h = hp * 2 + i
                            vblk = v_sb[:, (mj * HPC + h) * 65:(mj * HPC + h) * 65 + 65]
                            nc.tensor.matmul(
                                oT[i][:], vblk, pT[:, i * 512:(i + 1) * 512],
                                start=(mj == 0), stop=(mj == NT - 1),
                            )
                        drain_bg(1)
                    # Normalize: stash unnormalized output (frees the oT banks for
                    # the next chunk), fast-approx reciprocal of the row-sums,
                    # K=1 broadcast matmul, in-place scale, then ship the slice.
                    for i in range(2):
                        po = i * 64
                        ao_slice = ao_sb[t][po:po + 64, col:col + 512]
                        nc.vector.tensor_copy(ao_slice, oT[i][0:64, :])
                        r_row = small.tile([1, 512], F32, name="r_row", tag="r_row")
                        nc.vector.tensor_copy(r_row[:], oT[i][64:65, :])
                        rinv = small.tile([1, 512], F32, name="rinv", tag="rinv")
                        nc.vector.reciprocal_approx_fast(rinv[:], r_row[:])
                        rb_row = small.tile([1, 512], BF16, name="rb_row", tag="rb_row", bufs=4)
                        nc.vector.tensor_copy(rb_row[:], rinv[:])
                        rb_ps = psum.tile([64, 512], F32, name="rb_ps", tag="mm")
                        nc.tensor.matmul(rb_ps[:], ones_sb[:], rb_row[:], start=True, stop=True)
                        nc.vector.tensor_mul(ao_slice, ao_slice, rb_ps[:])
                        nc.gpsimd.dma_start(
                            ag_in[ci][t * 128 + po: t * 128 + po + 64, coff:coff + 512],
                            ao_slice,
                        )

            def emit_ag(ci):
                with nc.named_scope(f"ag{ci}"):
                    nc.gpsimd.collective_compute(
                        "AllGather",
                        mybir.AluOpType.bypass,
                        replica_groups=[[0, 1], [2, 3], [4, 5], [6, 7]],
                        ins=[ag_in[ci].opt()],
                        outs=[ag_out[ci].opt()],
                    )

            # ---- emission schedule ----
            # Prologue: just enough to start attention on (hp0, c0).
            qk_group(wq_sb, qT_sb[0], 0, 0)       # qT hp0 cols 0:512
            qk_group(wk_sb, kT_sb[0], 0, 0)       # kT hp0 m-tiles 0:4
            v_group(0)
            v_group(1)
            # k m-tiles 4..15 for hp0: needed from mj=4 on; DMA-gated anyway
            for ni in range(1, 4):
                qk_group(wk_sb, kT_sb[0], 0, ni)

            # Background queues per chunk, drained 1 group per mj iteration.
            bg.extend([lambda mj=mj: v_group(mj) for mj in range(2, NT)])
            attn_chunk(0, 0)

            bg.extend([lambda ni=ni: qk_group(wk_sb, kT_sb[1], 1, ni) for ni in range(4)])
            bg.extend([lambda ni=ni: qk_group(wq_sb, qT_sb[1], 1, ni) for ni in range(2)])
            qk_group(wq_sb, qT_sb[0], 0, 1)
            attn_chunk(0, 1)

            bg.extend([lambda ni=ni: qk_group(wk_sb, kT_sb[2], 2, ni) for ni in range(4)])
            bg.extend([lambda ni=ni: qk_group(wq_sb, qT_sb[2], 2, ni) for ni in range(2)])
            attn_chunk(1, 0)
            attn_chunk(1, 1)

            # q columns for chunks 2,3 of all head-pairs
            bg.extend([
                lambda hp=hp, ni=ni: qk_group(wq_sb, qT_sb[hp], hp, ni)
                for hp in range(3) for ni in range(2, 4)
            ])
            attn_chunk(2, 0)
            attn_chunk(2, 1)
            emit_ag(0)
            proj_load(0)

            bg.extend([lambda njl=njl: proj_group(0, njl) for njl in range(8)])
            attn_chunk(0, 2)
            attn_chunk(1, 2)
            attn_chunk(2, 2)
            emit_ag(1)
            proj_load(1)

            bg.extend([lambda njl=njl: proj_group(1, njl) for njl in range(4)])
            attn_chunk(0, 3)
            attn_chunk(1, 3)
            attn_chunk(2, 3)
            emit_ag(2)
            proj_load(2)
            for njl in range(4):
                proj_group(2, njl)

    nc.finalize()
    return nc


_NC = None
LAST_RESULTS = None


def _get_nc():
    global _NC
    if _NC is None:
        _NC = _build()
    return _NC


def kernel(x, w_qkv, w_out, b_out, _trace=False):
    global LAST_RESULTS
    nc = _get_nc()

    x = np.asarray(x, dtype=np.float32)
    w_qkv = np.asarray(w_qkv, dtype=np.float32)
    w_out = np.asarray(w_out, dtype=np.float32)
    b_out = np.asarray(b_out, dtype=np.float32)

    bf16 = ml_dtypes.bfloat16
    in_maps = []
    for c in range(8):
        b, g = c // 2, c % 2
        s = g * KC
        in_maps.append({
            "xT": np.ascontiguousarray(x[b].T).astype(bf16),
            "wq": np.ascontiguousarray(w_qkv[:, s:s + KC]).astype(bf16),
            "wk": np.ascontiguousarray(w_qkv[:, C + s:C + s + KC]).astype(bf16),
            "wv": np.ascontiguousarray(w_qkv[:, 2 * C + s:2 * C + s + KC]).astype(bf16),
            "wo": np.ascontiguousarray(w_out[:, s:s + KC]).astype(bf16),
            "bb": np.tile(b_out[s:s + KC], (128, 1)),
        })

    res = run_bass_kernel_spmd(nc, in_maps, core_ids=list(range(8)), trace=_trace)
    LAST_RESULTS = res

    out = np.empty((B, N, C), dtype=np.float32)
    for c in range(8):
        b, g = c // 2, c % 2
        out[b, :, g * KC:(g + 1) * KC] = res.results[c]["y"]
    return out
